# revision 14
# baseline (speedup 1.0000x reference)
"""Trainium2 Bass kernel for nn_MultiHeadAttention_79706003079680.

Reference (fp32):
    qp = (q @ Wq + bq) * SCALE      # [B, N, PROJ]
    kp = k @ Wk + bk
    vp = v @ Wv + bv
    scores = einsum('bnd,bmd->bnm', qp, kp)
    attn = softmax(scores, axis=1)          # over the QUERY axis n
    x = einsum('bnm,bmd->bnd', attn, vp)
    out = x @ Wo + bo                       # [B, N, HIDDEN]

Sharding: 8 cores = 4 batches x 2 key-halves (m in [mh*1024, mh*1024+1024)).
Softmax over n couples all queries for a fixed key m, so each core keeps
all n=2048 queries and a slice of keys. Each core emits a partial
out^T [HIDDEN, N]; the host sums the two key-halves per batch, transposes,
and adds bo.

On-chip layout (per core, P=128 partitions):
  qT   [P, 4, 2048] f32    q^T (h on partitions), via PE transpose
  qp_T [32, P, 2048] fp16  staged in DRAM (d on partitions)
  kp_T [P, 32, 1024] fp16  resident (d on partitions)
  vp   [8, P, 4096] bf16   staged in DRAM (m on partitions)
  attn [P, 8, 2048] bf16   resident, UNNORMALIZED exp(s - max_n s)
  softmax sums folded into vp rows as 1/sum[m] before the x matmuls.

All big matmuls run at 1 PE cycle/row: float32r (fp32 truncated to FP22)
for the projections, fp16/bf16 for scores / x / out.
"""

import numpy as np

import concourse.bass as bass
import concourse.mybir as mybir
import concourse.tile as tile
from concourse.bass_utils import run_bass_kernel_spmd
from concourse.masks import make_identity
from concourse.vector_clock import ScopedClock

P = 128
HIDDEN = 512
NUM_HEADS = 8
PROJ = NUM_HEADS * HIDDEN          # 4096
B, N = 4, 2048
M = N // 2                         # keys per core = 1024
SCALE = (HIDDEN // NUM_HEADS) ** -0.5

HB = HIDDEN // P                   # 4 h-blocks of 128
DB = PROJ // P                     # 32 d-blocks of 128
NB = N // 512                      # 4 n-blocks of 512
MB = M // P                        # 8 m-blocks of 128

F32 = mybir.dt.float32
F32R = mybir.dt.float32r
F16 = mybir.dt.float16
BF16 = mybir.dt.bfloat16
AX = mybir.AxisListType.X
AF = mybir.ActivationFunctionType


MAX_WAITS = 1


def split_excess_waits(nc, max_waits=MAX_WAITS):
    """Move excess per-instruction sem waits onto same-engine NoOps.

    This walrus build rejects instructions carrying more than a couple of
    sync-wait commands ("Too many sync wait commands" in setupSyncWait).
    A NoOp placed immediately before the instruction on the same engine
    enforces the wait in program order with identical semantics.
    """
    n_extra = 0
    for f in nc.m.functions:
        for bb in f.blocks:
            insts = bb.instructions
            i = 0
            while i < len(insts):
                inst = insts[i]
                si = getattr(inst, "sync_info", None)
                if si is not None and si.on_wait and len(si.on_wait) > max_waits:
                    waits = list(si.on_wait)
                    si.on_wait = waits[: max_waits]
                    for w in waits[max_waits:]:
                        n_extra += 1
                        nop = mybir.InstNoOp(
                            name=f"I-wsplit{n_extra}",
                            ins=[],
                            outs=[],
                            engine=inst.engine,
                        )
                        nop.sync_info = mybir.SyncInfo(on_wait=[w], on_update=[])
                        try:
                            nc.register_instruction(nop)
                        except Exception:
                            pass
                        # insert immediately before inst (inst shifts right)
                        insts.insert(i, nop)
                        i += 1
                i += 1
    return n_extra


class PatchedTC(tile.TileContext):
    """TileContext that post-processes the module to satisfy this walrus
    build's per-instruction sync-wait limit."""

    def __exit__(self, exc_type, exc_val, exc_tb):
        ret = super().__exit__(exc_type, exc_val, exc_tb)
        if exc_type is None:
            split_excess_waits(self.nc)
        return ret


def r(ap):
    return ap.bitcast(F32R)


def _phase_a1(nc, tc, pst, psm, qb, Wq, Wo, qp_d, wo_d, bqs, ident):
    """q transpose + qp_T projection -> DRAM fp16; Wo cast -> DRAM fp16."""
    with (
        tc.tile_pool(name="wfull", bufs=1) as wfull,
        tc.tile_pool(name="actT", bufs=1) as actT,
        tc.tile_pool(name="ldp", bufs=3) as ldp,
        tc.tile_pool(name="stp", bufs=4) as stp,
    ):
        qT = actT.tile([P, HB, N], F32, tag="qT")
        wq_s = wfull.tile([P, HB, PROJ], F32, tag="w")
        wq_src = Wq.ap().rearrange("(hb p) d -> p hb d", p=P).bitcast(F32R)
        for nt in range(N // P):
            q_t = ldp.tile([P, HIDDEN], F32, tag="ld")
            nc.sync.dma_start(out=q_t, in_=qb[nt * P : (nt + 1) * P, :])
            for hb in range(HB):
                pt = pst.tile([P, P], F32, tag="tp")
                nc.tensor.transpose(pt, q_t[:, hb * P : (hb + 1) * P], ident)
                nc.vector.tensor_copy(qT[:, hb, nt * P : (nt + 1) * P].bitcast(F32R), pt.bitcast(F32R))
        # d-sliced weight loads: first quarter lands early so the first
        # projection matmuls start without waiting for the full 8MB
        DQ = PROJ // 4
        for dsl in range(4):
            nc.sync.dma_start(
                out=wq_s[:, :, dsl * DQ : (dsl + 1) * DQ].bitcast(F32R),
                in_=wq_src[:, :, dsl * DQ : (dsl + 1) * DQ],
            )
        for db in range(DB):
            for nb in range(NB):
                ps = psm.tile([P, 512], F32, tag="mm")
                for hb in range(HB):
                    nc.tensor.matmul(
                        ps,
                        r(wq_s[:, hb, db * P : (db + 1) * P]),
                        r(qT[:, hb, nb * 512 : (nb + 1) * 512]),
                        start=(hb == 0),
                        stop=(hb == HB - 1),
                    )
                st = stp.tile([P, 512], F16, tag="st")
                nc.scalar.activation(
                    st, ps, AF.Identity, bias=bqs[:, db : db + 1], scale=SCALE
                )
                nc.sync.dma_start(out=qp_d[db, :, nb * 512 : (nb + 1) * 512], in_=st)


def _phase_a23(nc, tc, pst, psm, kb, vb, Wk, Wv, bv, vp_d, kpT, bks, ident):
    """k/v transposes, kp_T projection -> SBUF fp16, vp -> DRAM bf16."""
    with (
        tc.tile_pool(name="wfull2", bufs=1) as wfull2,
        tc.tile_pool(name="actT2", bufs=1) as actT2,
        tc.tile_pool(name="ldp2", bufs=3) as ldp2,
        tc.tile_pool(name="stp2", bufs=4) as stp2,
        tc.tile_pool(name="brow", bufs=1) as brow,
    ):
        wk_s = wfull2.tile([P, HB, PROJ], F32, tag="w")
        wk_src = Wk.ap().rearrange("(hb p) d -> p hb d", p=P).bitcast(F32R)
        DQ = PROJ // 4
        for dsl in range(4):
            nc.sync.dma_start(
                out=wk_s[:, :, dsl * DQ : (dsl + 1) * DQ].bitcast(F32R),
                in_=wk_src[:, :, dsl * DQ : (dsl + 1) * DQ],
            )
        kT = actT2.tile([P, HB, M], F32, tag="aT")
        for mt in range(M // P):
            k_t = ldp2.tile([P, HIDDEN], F32, tag="ld")
            nc.sync.dma_start(out=k_t, in_=kb[mt * P : (mt + 1) * P, :])
            for hb in range(HB):
                pt = pst.tile([P, P], F32, tag="tp")
                nc.tensor.transpose(pt, k_t[:, hb * P : (hb + 1) * P], ident)
                nc.vector.tensor_copy(kT[:, hb, mt * P : (mt + 1) * P].bitcast(F32R), pt.bitcast(F32R))
        for db in range(DB):
            for m2 in range(M // 512):
                ps = psm.tile([P, 512], F32, tag="mm")
                for hb in range(HB):
                    nc.tensor.matmul(
                        ps,
                        r(wk_s[:, hb, db * P : (db + 1) * P]),
                        r(kT[:, hb, m2 * 512 : (m2 + 1) * 512]),
                        start=(hb == 0),
                        stop=(hb == HB - 1),
                    )
                nc.scalar.activation(
                    kpT[:, db, m2 * 512 : (m2 + 1) * 512],
                    ps,
                    AF.Identity,
                    bias=bks[:, db : db + 1],
                    scale=1.0,
                )

        bvrow = brow.tile([1, PROJ], F32)
        nc.sync.dma_start(out=bvrow.bitcast(F32R), in_=bv.ap().rearrange("(o a) -> o a", o=1).bitcast(F32R))
        ones_tmp = brow.tile([1, P], F32)
        nc.vector.memset(ones_tmp, 1.0)
        ones_row = brow.tile([1, P], F32)
        nc.vector.tensor_copy(ones_row.bitcast(F32R), ones_tmp.bitcast(F32R))
        vT = actT2.tile([P, HB, M], F32, tag="aT")
        for mt in range(M // P):
            v_t = ldp2.tile([P, HIDDEN], F32, tag="ld")
            nc.sync.dma_start(out=v_t, in_=vb[mt * P : (mt + 1) * P, :])
            for hb in range(HB):
                pt = pst.tile([P, P], F32, tag="tp")
                nc.tensor.transpose(pt, v_t[:, hb * P : (hb + 1) * P], ident)
                nc.vector.tensor_copy(vT[:, hb, mt * P : (mt + 1) * P].bitcast(F32R), pt.bitcast(F32R))
        wv_s = wfull2.tile([P, HB, PROJ], F32, tag="w")
        wv_src = Wv.ap().rearrange("(hb p) d -> p hb d", p=P).bitcast(F32R)
        for dsl in range(4):
            nc.sync.dma_start(
                out=wv_s[:, :, dsl * DQ : (dsl + 1) * DQ].bitcast(F32R),
                in_=wv_src[:, :, dsl * DQ : (dsl + 1) * DQ],
            )
        for mb in range(MB):
            for ds in range(PROJ // 512):
                ps = psm.tile([P, 512], F32, tag="mm")
                for hb in range(HB):
                    nc.tensor.matmul(
                        ps,
                        r(vT[:, hb, mb * P : (mb + 1) * P]),
                        r(wv_s[:, hb, ds * 512 : (ds + 1) * 512]),
                        start=(hb == 0),
                        stop=False,
                    )
                nc.tensor.matmul(
                    ps,
                    r(ones_row),
                    r(bvrow[:, ds * 512 : (ds + 1) * 512]),
                    start=False,
                    stop=True,
                )
                st = stp2.tile([P, 512], BF16, tag="st")
                nc.vector.tensor_copy(st, ps)
                nc.sync.dma_start(out=vp_d[mb, :, ds * 512 : (ds + 1) * 512], in_=st)


def _phase_b(nc, tc, psm, qp_d, kpT, attn, rsum, rinv, Wo, wo_d):
    """scores_T = kp_T^T(d) qp_T per m-block; softmax over free axis n.

    The Wo fp32->fp16 cast rides along here: phase B's DMA load is light
    (qp_s readback), so the 12MB of Wo traffic hides under the scores
    matmuls instead of competing with the startup q/Wq loads."""
    with (
        tc.tile_pool(name="qps", bufs=2) as qps,
        tc.tile_pool(name="scp", bufs=1) as scp,
        tc.tile_pool(name="nmx", bufs=4) as nmx,
        tc.tile_pool(name="wol", bufs=3) as wol,
        tc.tile_pool(name="wos", bufs=3) as wos,
    ):
        for db in range(DB):
            wo_t = wol.tile([P, HIDDEN], F32, tag="wl")
            nc.sync.dma_start(out=wo_t, in_=Wo[db * P : (db + 1) * P, :])
            wo_c = wos.tile([P, HIDDEN], F16, tag="ws")
            nc.vector.tensor_copy(wo_c, wo_t)
            nc.sync.dma_start(out=wo_d[db], in_=wo_c)
        scores = scp.tile([P, 4, N], F32)
        for mh2 in range(2):
            for nb in range(NB):
                qp_s = qps.tile([P, DB, 512], F16, tag="qp")
                nc.sync.dma_start(
                    out=qp_s,
                    in_=qp_d[:, :, nb * 512 : (nb + 1) * 512].rearrange(
                        "db p n -> p db n"
                    ),
                )
                for mbl in range(4):
                    mb = mh2 * 4 + mbl
                    ps = psm.tile([P, 512], F32, tag="mm")
                    for db in range(DB):
                        nc.tensor.matmul(
                            ps,
                            kpT[:, db, mb * P : (mb + 1) * P],
                            qp_s[:, db, :],
                            start=(db == 0),
                            stop=(db == DB - 1),
                        )
                    nc.vector.tensor_copy(
                        scores[:, mbl, nb * 512 : (nb + 1) * 512], ps
                    )
            for mbl in range(4):
                mb = mh2 * 4 + mbl
                negmax = nmx.tile([P, 1], F32, tag="nm")
                nc.vector.reduce_max(negmax, scores[:, mbl, :], axis=AX, negate=True)
                nc.scalar.activation(
                    attn[:, mb, :],
                    scores[:, mbl, :],
                    AF.Exp,
                    bias=negmax,
                    scale=1.0,
                    accum_out=rsum[:, mb : mb + 1],
                )
                nc.vector.reciprocal(rinv[:, mb : mb + 1], rsum[:, mb : mb + 1])
                nc.vector.tensor_scalar_mul(
                    attn[:, mb, :], attn[:, mb, :], rinv[:, mb : mb + 1]
                )


def _phase_c(nc, tc, psm, vp_d, wo_d, attn, outT):
    """x_T = vp^T @ attn (per d-block), out_T = Wo^T @ x_T."""
    with (
        tc.tile_pool(name="vpc", bufs=1) as vpc,
        tc.tile_pool(name="woc", bufs=1) as woc,
        tc.tile_pool(name="xsp", bufs=2) as xsp,
        tc.tile_pool(name="osp", bufs=2) as osp,
    ):
        vp_c = vpc.tile([P, MB, PROJ], BF16)
        for mb in range(MB):
            nc.sync.dma_start(out=vp_c[:, mb, :], in_=vp_d[mb])
        wo16 = woc.tile([P, DB, HIDDEN], F16)
        nc.sync.dma_start(out=wo16, in_=wo_d.ap().rearrange("db p h -> p db h"))
        for nb in range(NB):
            x_s = xsp.tile([P, DB, 512], F16, tag="x")
            for db in range(DB):
                ps = psm.tile([P, 512], F32, tag="mm")
                for mch in range(MB):
                    nc.tensor.matmul(
                        ps,
                        vp_c[:, mch, db * P : (db + 1) * P],
                        attn[:, mch, nb * 512 : (nb + 1) * 512],
                        start=(mch == 0),
                        stop=(mch == MB - 1),
                    )
                nc.vector.tensor_copy(x_s[:, db, :], ps)
            for hb in range(HB):
                ps2 = psm.tile([P, 512], F32, tag="mm")
                for db in range(DB):
                    nc.tensor.matmul(
                        ps2,
                        wo16[:, db, hb * P : (hb + 1) * P],
                        x_s[:, db, :],
                        start=(db == 0),
                        stop=(db == DB - 1),
                    )
                ot = osp.tile([P, 512], F32, tag="ot")
                nc.vector.tensor_copy(ot, ps2)
                nc.sync.dma_start(
                    out=outT[hb * P : (hb + 1) * P, nb * 512 : (nb + 1) * 512],
                    in_=ot,
                )


def build_nc():
    nc = bass.Bass("TRN2", target_bir_lowering=False, debug=False, num_devices=8)

    qb = nc.dram_tensor("qb", [N, HIDDEN], F32, kind="ExternalInput")
    kb = nc.dram_tensor("kb", [M, HIDDEN], F32, kind="ExternalInput")
    vb = nc.dram_tensor("vb", [M, HIDDEN], F32, kind="ExternalInput")
    Wq = nc.dram_tensor("Wq", [HIDDEN, PROJ], F32, kind="ExternalInput")
    Wk = nc.dram_tensor("Wk", [HIDDEN, PROJ], F32, kind="ExternalInput")
    Wv = nc.dram_tensor("Wv", [HIDDEN, PROJ], F32, kind="ExternalInput")
    Wo = nc.dram_tensor("Wo", [PROJ, HIDDEN], F32, kind="ExternalInput")
    bq = nc.dram_tensor("bq", [PROJ], F32, kind="ExternalInput")
    bk = nc.dram_tensor("bk", [PROJ], F32, kind="ExternalInput")
    bv = nc.dram_tensor("bv", [PROJ], F32, kind="ExternalInput")
    outT = nc.dram_tensor("outT", [HIDDEN, N], F32, kind="ExternalOutput")

    qp_d = nc.dram_tensor("qp_d", [DB, P, N], F16, kind="Internal")
    vp_d = nc.dram_tensor("vp_d", [MB, P, PROJ], BF16, kind="Internal")
    wo_d = nc.dram_tensor("wo_d", [DB, P, HIDDEN], F16, kind="Internal")

    with PatchedTC(nc) as tc:
        with (
            tc.tile_pool(name="singles", bufs=1) as singles,
            tc.tile_pool(name="pst", bufs=4, space="PSUM") as pst,
            tc.tile_pool(name="psm", bufs=4, space="PSUM") as psm,
        ):
            ident = singles.tile([P, P], F32)
            make_identity(nc, ident)
            bqs = singles.tile([P, DB], F32)
            nc.sync.dma_start(out=bqs, in_=bq.ap().rearrange("(a b) -> b a", b=P))
            nc.scalar.mul(bqs, bqs, SCALE)
            bks = singles.tile([P, DB], F32)
            nc.sync.dma_start(out=bks, in_=bk.ap().rearrange("(a b) -> b a", b=P))
            rsum = singles.tile([P, MB], F32)
            rinv = singles.tile([P, MB], F32)

            _phase_a1(nc, tc, pst, psm, qb, Wq, Wo, qp_d, wo_d, bqs, ident)

            # attn outlives kpT: open its pool first, allocate lazily.
            with tc.tile_pool(name="attnp", bufs=1) as attnp:
                with tc.tile_pool(name="kpTp", bufs=1) as kpTp:
                    kpT = kpTp.tile([P, DB, M], F16)
                    _phase_a23(
                        nc, tc, pst, psm, kb, vb, Wk, Wv, bv, vp_d, kpT, bks, ident
                    )
                    attn = attnp.tile([P, MB, N], BF16)
                    _phase_b(nc, tc, psm, qp_d, kpT, attn, rsum, rinv, Wo, wo_d)
                _phase_c(nc, tc, psm, vp_d, wo_d, attn, outT)
    # A handful of waits are attached after the TileContext's own exit
    # processing; sweep again until the module is clean.
    while split_excess_waits(nc):
        pass
    return nc


class _Runner:
    """Compile the Bass program once; re-execute cheaply on later calls.

    Mirrors bass2jax.run_bass_via_pjrt's multi-core path, but keeps the
    jitted shard_map callable so repeated kernel() calls skip the
    multi-minute neuronxcc compile.
    """

    def __init__(self):
        import jax
        from jax.sharding import Mesh, PartitionSpec
        from jax.experimental.shard_map import shard_map
        from concourse import bass2jax
        import concourse.mybir as mb

        self.jax = jax
        nc = build_nc()
        self.nc = nc
        bass2jax.install_neuronx_cc_hook()

        in_names, out_names, out_avals, zero_outs = [], [], [], []
        partition_name = (
            nc.partition_id_tensor.name if nc.partition_id_tensor else None
        )
        for alloc in nc.m.functions[0].allocations:
            if not isinstance(alloc, mb.MemoryLocationSet):
                continue
            name = alloc.memorylocations[0].name
            if alloc.kind == "ExternalInput":
                if name != partition_name:
                    in_names.append(name)
            elif alloc.kind == "ExternalOutput":
                shape = tuple(alloc.tensor_shape)
                dtype = mb.dt.np(alloc.dtype)
                out_names.append(name)
                out_avals.append(jax.core.ShapedArray(shape, dtype))
                zero_outs.append(np.zeros(shape, dtype))
        n_params = len(in_names)
        n_outs = len(out_avals)
        all_in_names = list(in_names) + list(out_names)
        if partition_name is not None:
            all_in_names.append(partition_name)
        self.in_names = in_names
        self.out_names = out_names
        self.zero_outs = zero_outs

        def _body(*args):
            operands = list(args)
            if partition_name is not None:
                operands.append(bass2jax.partition_id_tensor())
            outs = bass2jax._bass_exec_p.bind(
                *operands,
                out_avals=tuple(out_avals),
                in_names=tuple(all_in_names),
                out_names=tuple(out_names),
                lowering_input_output_aliases=(),
                sim_require_finite=True,
                sim_require_nnan=True,
                nc=nc,
            )
            return tuple(outs)

        devices = jax.devices()[:8]
        mesh = Mesh(np.asarray(devices), ("core",))
        self.mesh = mesh
        in_specs = (PartitionSpec("core"),) * (n_params + n_outs)
        out_specs = (PartitionSpec("core"),) * n_outs
        self.body = _body
        self.in_specs = in_specs
        self.out_specs = out_specs
        donate = tuple(range(n_params, n_params + n_outs))
        self.sharded = jax.jit(
            shard_map(
                _body,
                mesh=mesh,
                in_specs=in_specs,
                out_specs=out_specs,
                check_rep=False,
            ),
            donate_argnums=donate,
            keep_unused=True,
        )
        self.out_avals = out_avals

    def prepare(self, in_maps):
        """Concatenate per-core inputs along axis 0 (device-shardable)."""
        return [
            np.concatenate([in_maps[c][name] for c in range(8)], axis=0)
            for name in self.in_names
        ]

    def run(self, concat_in):
        zeros = [
            np.zeros((8 * z.shape[0], *z.shape[1:]), z.dtype) for z in self.zero_outs
        ]
        out_arrs = self.sharded(*concat_in, *zeros)
        res = []
        for c in range(8):
            res.append(
                {
                    name: np.asarray(out_arrs[i]).reshape(
                        8, *self.out_avals[i].shape
                    )[c]
                    for i, name in enumerate(self.out_names)
                }
            )
        return res


_RUNNER = None


def _get_runner():
    global _RUNNER
    if _RUNNER is None:
        _RUNNER = _Runner()
    return _RUNNER


def make_in_maps(inputs):
    f32 = lambda x: np.ascontiguousarray(np.asarray(x, dtype=np.float32))
    q, k, v = f32(inputs["q"]), f32(inputs["k"]), f32(inputs["v"])
    Wq, Wk, Wv, Wo = (f32(inputs[n]) for n in ("Wq", "Wk", "Wv", "Wo"))
    bq, bk, bv = (f32(inputs[n]) for n in ("bq", "bk", "bv"))
    in_maps = []
    for c in range(8):
        b, mh = c // 2, c % 2
        sl = slice(mh * M, (mh + 1) * M)
        in_maps.append(
            {
                "qb": q[b],
                "kb": np.ascontiguousarray(k[b, sl]),
                "vb": np.ascontiguousarray(v[b, sl]),
                "Wq": Wq, "Wk": Wk, "Wv": Wv, "Wo": Wo,
                "bq": bq, "bk": bk, "bv": bv,
            }
        )
    return in_maps


def assemble_out(results, bo):
    out = np.empty((B, N, HIDDEN), dtype=np.float32)
    for b in range(B):
        acc = results[2 * b]["outT"] + results[2 * b + 1]["outT"]
        out[b] = acc.T + bo[None, :]
    return out


def kernel(**inputs):
    runner = _get_runner()
    res = runner.run(runner.prepare(make_in_maps(inputs)))
    bo = np.asarray(inputs["bo"], dtype=np.float32)
    return assemble_out(res, bo)


# revision 15
# speedup vs baseline: 1.3056x; 1.3056x over previous
"""Trainium2 Bass kernel for nn_MultiHeadAttention_79706003079680.

Reference (fp32):
    qp = (q @ Wq + bq) * SCALE      # [B, N, PROJ]
    kp = k @ Wk + bk
    vp = v @ Wv + bv
    scores = einsum('bnd,bmd->bnm', qp, kp)
    attn = softmax(scores, axis=1)          # over the QUERY axis n
    x = einsum('bnm,bmd->bnd', attn, vp)
    out = x @ Wo + bo                       # [B, N, HIDDEN]

Sharding: 8 cores = 4 batches x 2 key-halves (m in [mh*1024, mh*1024+1024)).
Softmax over n couples all queries for a fixed key m, so each core keeps
all n=2048 queries and a slice of keys. Each core emits a partial
out^T [HIDDEN, N]; the host sums the two key-halves per batch, transposes,
and adds bo.

On-chip layout (per core, P=128 partitions):
  qT   [P, 4, 2048] f32    q^T (h on partitions), via PE transpose
  qp_T [32, P, 2048] fp16  staged in DRAM (d on partitions)
  kp_T [P, 32, 1024] fp16  resident (d on partitions)
  vp   [8, P, 4096] bf16   staged in DRAM (m on partitions)
  attn [P, 8, 2048] bf16   resident, normalized softmax weights

All big matmuls run at 1 PE cycle/row: float32r (fp32 truncated to FP22)
for the projections, fp16/bf16 for scores / x / out.
"""

import numpy as np

import concourse.bass as bass
import concourse.mybir as mybir
import concourse.tile as tile
from concourse.masks import make_identity

P = 128
HIDDEN = 512
NUM_HEADS = 8
PROJ = NUM_HEADS * HIDDEN          # 4096
B, N = 4, 2048
M = N // 2                         # keys per core = 1024
SCALE = (HIDDEN // NUM_HEADS) ** -0.5

HB = HIDDEN // P                   # 4 h-blocks of 128
DB = PROJ // P                     # 32 d-blocks of 128
NB = N // 512                      # 4 n-blocks of 512
MB = M // P                        # 8 m-blocks of 128

F32 = mybir.dt.float32
F32R = mybir.dt.float32r
F16 = mybir.dt.float16
BF16 = mybir.dt.bfloat16
AX = mybir.AxisListType.X
AF = mybir.ActivationFunctionType


MAX_WAITS = 1


def split_excess_waits(nc, max_waits=MAX_WAITS):
    """Move excess per-instruction sem waits onto same-engine NoOps.

    This walrus build rejects instructions carrying more than a couple of
    sync-wait commands ("Too many sync wait commands" in setupSyncWait).
    A NoOp placed immediately before the instruction on the same engine
    enforces the wait in program order with identical semantics.
    """
    n_extra = 0
    for f in nc.m.functions:
        for bb in f.blocks:
            insts = bb.instructions
            i = 0
            while i < len(insts):
                inst = insts[i]
                si = getattr(inst, "sync_info", None)
                if si is not None and si.on_wait and len(si.on_wait) > max_waits:
                    waits = list(si.on_wait)
                    si.on_wait = waits[: max_waits]
                    for w in waits[max_waits:]:
                        n_extra += 1
                        nop = mybir.InstNoOp(
                            name=f"I-wsplit{n_extra}",
                            ins=[],
                            outs=[],
                            engine=inst.engine,
                        )
                        nop.sync_info = mybir.SyncInfo(on_wait=[w], on_update=[])
                        try:
                            nc.register_instruction(nop)
                        except Exception:
                            pass
                        # insert immediately before inst (inst shifts right)
                        insts.insert(i, nop)
                        i += 1
                i += 1
    return n_extra


class PatchedTC(tile.TileContext):
    """TileContext that post-processes the module to satisfy this walrus
    build's per-instruction sync-wait limit."""

    def __exit__(self, exc_type, exc_val, exc_tb):
        ret = super().__exit__(exc_type, exc_val, exc_tb)
        if exc_type is None:
            split_excess_waits(self.nc)
        return ret


def r(ap):
    return ap.bitcast(F32R)


def _phase_a1(nc, tc, pst, psm, qb, Wq, Wo, qp_d, wo_d, bqs, ident):
    """q transpose + qp_T projection -> DRAM fp16; Wo cast -> DRAM fp16."""
    with (
        tc.tile_pool(name="wfull", bufs=1) as wfull,
        tc.tile_pool(name="actT", bufs=1) as actT,
        tc.tile_pool(name="ldp", bufs=3) as ldp,
        tc.tile_pool(name="stp", bufs=4) as stp,
    ):
        qT = actT.tile([P, HB, N], F32, tag="qT")
        wq_s = wfull.tile([P, HB, PROJ], F32, tag="w")
        wq_src = Wq.ap().rearrange("(hb p) d -> p hb d", p=P).bitcast(F32R)
        for nt in range(N // P):
            q_t = ldp.tile([P, HIDDEN], F32, tag="ld")
            nc.sync.dma_start(out=q_t, in_=qb[nt * P : (nt + 1) * P, :])
            for hb in range(HB):
                pt = pst.tile([P, P], F32, tag="tp")
                nc.tensor.transpose(pt, q_t[:, hb * P : (hb + 1) * P], ident)
                nc.vector.tensor_copy(qT[:, hb, nt * P : (nt + 1) * P].bitcast(F32R), pt.bitcast(F32R))
        # d-sliced weight loads: first quarter lands early so the first
        # projection matmuls start without waiting for the full 8MB
        DQ = PROJ // 4
        for dsl in range(4):
            nc.sync.dma_start(
                out=wq_s[:, :, dsl * DQ : (dsl + 1) * DQ].bitcast(F32R),
                in_=wq_src[:, :, dsl * DQ : (dsl + 1) * DQ],
            )
        for db in range(DB):
            for nb in range(NB):
                ps = psm.tile([P, 512], F32, tag="mm")
                for hb in range(HB):
                    nc.tensor.matmul(
                        ps,
                        r(wq_s[:, hb, db * P : (db + 1) * P]),
                        r(qT[:, hb, nb * 512 : (nb + 1) * 512]),
                        start=(hb == 0),
                        stop=(hb == HB - 1),
                    )
                st = stp.tile([P, 512], F16, tag="st")
                nc.scalar.activation(
                    st, ps, AF.Identity, bias=bqs[:, db : db + 1], scale=SCALE
                )
                nc.sync.dma_start(out=qp_d[db, :, nb * 512 : (nb + 1) * 512], in_=st)


def _phase_a23(nc, tc, pst, psm, kb, vb, Wk, Wv, bv, vp_d, kpT, bks, ident):
    """k/v transposes, kp_T projection -> SBUF fp16, vp -> DRAM bf16."""
    with (
        tc.tile_pool(name="wfull2", bufs=1) as wfull2,
        tc.tile_pool(name="actT2", bufs=1) as actT2,
        tc.tile_pool(name="ldp2", bufs=3) as ldp2,
        tc.tile_pool(name="stp2", bufs=4) as stp2,
        tc.tile_pool(name="brow", bufs=1) as brow,
    ):
        wk_s = wfull2.tile([P, HB, PROJ], F32, tag="w")
        wk_src = Wk.ap().rearrange("(hb p) d -> p hb d", p=P).bitcast(F32R)
        DQ = PROJ // 4
        for dsl in range(4):
            nc.sync.dma_start(
                out=wk_s[:, :, dsl * DQ : (dsl + 1) * DQ].bitcast(F32R),
                in_=wk_src[:, :, dsl * DQ : (dsl + 1) * DQ],
            )
        kT = actT2.tile([P, HB, M], F32, tag="aT")
        for mt in range(M // P):
            k_t = ldp2.tile([P, HIDDEN], F32, tag="ld")
            nc.sync.dma_start(out=k_t, in_=kb[mt * P : (mt + 1) * P, :])
            for hb in range(HB):
                pt = pst.tile([P, P], F32, tag="tp")
                nc.tensor.transpose(pt, k_t[:, hb * P : (hb + 1) * P], ident)
                nc.vector.tensor_copy(kT[:, hb, mt * P : (mt + 1) * P].bitcast(F32R), pt.bitcast(F32R))
        for db in range(DB):
            for m2 in range(M // 512):
                ps = psm.tile([P, 512], F32, tag="mm")
                for hb in range(HB):
                    nc.tensor.matmul(
                        ps,
                        r(wk_s[:, hb, db * P : (db + 1) * P]),
                        r(kT[:, hb, m2 * 512 : (m2 + 1) * 512]),
                        start=(hb == 0),
                        stop=(hb == HB - 1),
                    )
                nc.scalar.activation(
                    kpT[:, db, m2 * 512 : (m2 + 1) * 512],
                    ps,
                    AF.Identity,
                    bias=bks[:, db : db + 1],
                    scale=1.0,
                )

        bvrow = brow.tile([1, PROJ], F32)
        nc.sync.dma_start(out=bvrow.bitcast(F32R), in_=bv.ap().rearrange("(o a) -> o a", o=1).bitcast(F32R))
        ones_tmp = brow.tile([1, P], F32)
        nc.vector.memset(ones_tmp, 1.0)
        ones_row = brow.tile([1, P], F32)
        nc.vector.tensor_copy(ones_row.bitcast(F32R), ones_tmp.bitcast(F32R))
        vT = actT2.tile([P, HB, M], F32, tag="aT")
        for mt in range(M // P):
            v_t = ldp2.tile([P, HIDDEN], F32, tag="ld")
            nc.sync.dma_start(out=v_t, in_=vb[mt * P : (mt + 1) * P, :])
            for hb in range(HB):
                pt = pst.tile([P, P], F32, tag="tp")
                nc.tensor.transpose(pt, v_t[:, hb * P : (hb + 1) * P], ident)
                nc.vector.tensor_copy(vT[:, hb, mt * P : (mt + 1) * P].bitcast(F32R), pt.bitcast(F32R))
        wv_s = wfull2.tile([P, HB, PROJ], F32, tag="w")
        wv_src = Wv.ap().rearrange("(hb p) d -> p hb d", p=P).bitcast(F32R)
        for dsl in range(4):
            nc.sync.dma_start(
                out=wv_s[:, :, dsl * DQ : (dsl + 1) * DQ].bitcast(F32R),
                in_=wv_src[:, :, dsl * DQ : (dsl + 1) * DQ],
            )
        for mb in range(MB):
            for ds in range(PROJ // 512):
                ps = psm.tile([P, 512], F32, tag="mm")
                for hb in range(HB):
                    nc.tensor.matmul(
                        ps,
                        r(vT[:, hb, mb * P : (mb + 1) * P]),
                        r(wv_s[:, hb, ds * 512 : (ds + 1) * 512]),
                        start=(hb == 0),
                        stop=False,
                    )
                nc.tensor.matmul(
                    ps,
                    r(ones_row),
                    r(bvrow[:, ds * 512 : (ds + 1) * 512]),
                    start=False,
                    stop=True,
                )
                st = stp2.tile([P, 512], BF16, tag="st")
                nc.vector.tensor_copy(st, ps)
                nc.sync.dma_start(out=vp_d[mb, :, ds * 512 : (ds + 1) * 512], in_=st)


def _phase_b(nc, tc, psm, qp_d, kpT, attn, rsum, rinv, Wo, wo_d):
    """scores_T = kp_T^T(d) qp_T per m-block; softmax over free axis n.

    The Wo fp32->fp16 cast rides along here: phase B's DMA load is light
    (qp_s readback), so the 12MB of Wo traffic hides under the scores
    matmuls instead of competing with the startup q/Wq loads."""
    with (
        tc.tile_pool(name="qps", bufs=2) as qps,
        tc.tile_pool(name="scp", bufs=1) as scp,
        tc.tile_pool(name="nmx", bufs=4) as nmx,
        tc.tile_pool(name="wol", bufs=3) as wol,
        tc.tile_pool(name="wos", bufs=3) as wos,
    ):
        for db in range(DB):
            wo_t = wol.tile([P, HIDDEN], F32, tag="wl")
            nc.sync.dma_start(out=wo_t, in_=Wo[db * P : (db + 1) * P, :])
            wo_c = wos.tile([P, HIDDEN], F16, tag="ws")
            nc.vector.tensor_copy(wo_c, wo_t)
            nc.sync.dma_start(out=wo_d[db], in_=wo_c)
        scores = scp.tile([P, 4, N], F32)
        for mh2 in range(2):
            for nb in range(NB):
                qp_s = qps.tile([P, DB, 512], F16, tag="qp")
                nc.sync.dma_start(
                    out=qp_s,
                    in_=qp_d[:, :, nb * 512 : (nb + 1) * 512].rearrange(
                        "db p n -> p db n"
                    ),
                )
                for mbl in range(4):
                    mb = mh2 * 4 + mbl
                    ps = psm.tile([P, 512], F32, tag="mm")
                    for db in range(DB):
                        nc.tensor.matmul(
                            ps,
                            kpT[:, db, mb * P : (mb + 1) * P],
                            qp_s[:, db, :],
                            start=(db == 0),
                            stop=(db == DB - 1),
                        )
                    nc.vector.tensor_copy(
                        scores[:, mbl, nb * 512 : (nb + 1) * 512], ps
                    )
            for mbl in range(4):
                mb = mh2 * 4 + mbl
                negmax = nmx.tile([P, 1], F32, tag="nm")
                nc.vector.reduce_max(negmax, scores[:, mbl, :], axis=AX, negate=True)
                nc.scalar.activation(
                    attn[:, mb, :],
                    scores[:, mbl, :],
                    AF.Exp,
                    bias=negmax,
                    scale=1.0,
                    accum_out=rsum[:, mb : mb + 1],
                )
                nc.vector.reciprocal(rinv[:, mb : mb + 1], rsum[:, mb : mb + 1])
                nc.vector.tensor_scalar_mul(
                    attn[:, mb, :], attn[:, mb, :], rinv[:, mb : mb + 1]
                )


def _phase_c(nc, tc, psm, vp_d, wo_d, attn, outT):
    """x_T = vp^T @ attn (per d-block), out_T = Wo^T @ x_T."""
    with (
        tc.tile_pool(name="vpc", bufs=1) as vpc,
        tc.tile_pool(name="woc", bufs=1) as woc,
        tc.tile_pool(name="xsp", bufs=2) as xsp,
        tc.tile_pool(name="osp", bufs=2) as osp,
    ):
        vp_c = vpc.tile([P, MB, PROJ], BF16)
        for mb in range(MB):
            nc.sync.dma_start(out=vp_c[:, mb, :], in_=vp_d[mb])
        wo16 = woc.tile([P, DB, HIDDEN], F16)
        nc.sync.dma_start(out=wo16, in_=wo_d.ap().rearrange("db p h -> p db h"))
        for nb in range(NB):
            x_s = xsp.tile([P, DB, 512], F16, tag="x")
            for db in range(DB):
                ps = psm.tile([P, 512], F32, tag="mm")
                for mch in range(MB):
                    nc.tensor.matmul(
                        ps,
                        vp_c[:, mch, db * P : (db + 1) * P],
                        attn[:, mch, nb * 512 : (nb + 1) * 512],
                        start=(mch == 0),
                        stop=(mch == MB - 1),
                    )
                nc.vector.tensor_copy(x_s[:, db, :], ps)
            for hb in range(HB):
                ps2 = psm.tile([P, 512], F32, tag="mm")
                for db in range(DB):
                    nc.tensor.matmul(
                        ps2,
                        wo16[:, db, hb * P : (hb + 1) * P],
                        x_s[:, db, :],
                        start=(db == 0),
                        stop=(db == DB - 1),
                    )
                ot = osp.tile([P, 512], F32, tag="ot")
                nc.vector.tensor_copy(ot, ps2)
                nc.sync.dma_start(
                    out=outT[hb * P : (hb + 1) * P, nb * 512 : (nb + 1) * 512],
                    in_=ot,
                )


def build_nc():
    nc = bass.Bass("TRN2", target_bir_lowering=False, debug=False, num_devices=8)

    qb = nc.dram_tensor("qb", [N, HIDDEN], F32, kind="ExternalInput")
    kb = nc.dram_tensor("kb", [M, HIDDEN], F32, kind="ExternalInput")
    vb = nc.dram_tensor("vb", [M, HIDDEN], F32, kind="ExternalInput")
    Wq = nc.dram_tensor("Wq", [HIDDEN, PROJ], F32, kind="ExternalInput")
    Wk = nc.dram_tensor("Wk", [HIDDEN, PROJ], F32, kind="ExternalInput")
    Wv = nc.dram_tensor("Wv", [HIDDEN, PROJ], F32, kind="ExternalInput")
    Wo = nc.dram_tensor("Wo", [PROJ, HIDDEN], F32, kind="ExternalInput")
    bq = nc.dram_tensor("bq", [PROJ], F32, kind="ExternalInput")
    bk = nc.dram_tensor("bk", [PROJ], F32, kind="ExternalInput")
    bv = nc.dram_tensor("bv", [PROJ], F32, kind="ExternalInput")
    outT = nc.dram_tensor("outT", [HIDDEN, N], F32, kind="ExternalOutput")

    qp_d = nc.dram_tensor("qp_d", [DB, P, N], F16, kind="Internal")
    vp_d = nc.dram_tensor("vp_d", [MB, P, PROJ], BF16, kind="Internal")
    wo_d = nc.dram_tensor("wo_d", [DB, P, HIDDEN], F16, kind="Internal")

    with PatchedTC(nc) as tc:
        with (
            tc.tile_pool(name="singles", bufs=1) as singles,
            tc.tile_pool(name="pst", bufs=4, space="PSUM") as pst,
            tc.tile_pool(name="psm", bufs=4, space="PSUM") as psm,
        ):
            ident = singles.tile([P, P], F32)
            make_identity(nc, ident)
            bqs = singles.tile([P, DB], F32)
            nc.sync.dma_start(out=bqs, in_=bq.ap().rearrange("(a b) -> b a", b=P))
            nc.scalar.mul(bqs, bqs, SCALE)
            bks = singles.tile([P, DB], F32)
            nc.sync.dma_start(out=bks, in_=bk.ap().rearrange("(a b) -> b a", b=P))
            rsum = singles.tile([P, MB], F32)
            rinv = singles.tile([P, MB], F32)

            _phase_a1(nc, tc, pst, psm, qb, Wq, Wo, qp_d, wo_d, bqs, ident)

            # attn outlives kpT: open its pool first, allocate lazily.
            with tc.tile_pool(name="attnp", bufs=1) as attnp:
                with tc.tile_pool(name="kpTp", bufs=1) as kpTp:
                    kpT = kpTp.tile([P, DB, M], F16)
                    _phase_a23(
                        nc, tc, pst, psm, kb, vb, Wk, Wv, bv, vp_d, kpT, bks, ident
                    )
                    attn = attnp.tile([P, MB, N], BF16)
                    _phase_b(nc, tc, psm, qp_d, kpT, attn, rsum, rinv, Wo, wo_d)
                _phase_c(nc, tc, psm, vp_d, wo_d, attn, outT)
    # A handful of waits are attached after the TileContext's own exit
    # processing; sweep again until the module is clean.
    while split_excess_waits(nc):
        pass
    return nc


class _Runner:
    """Compile the Bass program once; re-execute cheaply on later calls.

    Mirrors bass2jax.run_bass_via_pjrt's multi-core path, but keeps the
    jitted shard_map callable so repeated kernel() calls skip the
    multi-minute neuronxcc compile.
    """

    def __init__(self):
        import jax
        from jax.sharding import Mesh, PartitionSpec
        from jax.experimental.shard_map import shard_map
        from concourse import bass2jax
        import concourse.mybir as mb

        self.jax = jax
        nc = build_nc()
        self.nc = nc
        bass2jax.install_neuronx_cc_hook()

        in_names, out_names, out_avals, zero_outs = [], [], [], []
        partition_name = (
            nc.partition_id_tensor.name if nc.partition_id_tensor else None
        )
        for alloc in nc.m.functions[0].allocations:
            if not isinstance(alloc, mb.MemoryLocationSet):
                continue
            name = alloc.memorylocations[0].name
            if alloc.kind == "ExternalInput":
                if name != partition_name:
                    in_names.append(name)
            elif alloc.kind == "ExternalOutput":
                shape = tuple(alloc.tensor_shape)
                dtype = mb.dt.np(alloc.dtype)
                out_names.append(name)
                out_avals.append(jax.core.ShapedArray(shape, dtype))
                zero_outs.append(np.zeros(shape, dtype))
        n_params = len(in_names)
        n_outs = len(out_avals)
        all_in_names = list(in_names) + list(out_names)
        if partition_name is not None:
            all_in_names.append(partition_name)
        self.in_names = in_names
        self.out_names = out_names
        self.zero_outs = zero_outs

        def _body(*args):
            operands = list(args)
            if partition_name is not None:
                operands.append(bass2jax.partition_id_tensor())
            outs = bass2jax._bass_exec_p.bind(
                *operands,
                out_avals=tuple(out_avals),
                in_names=tuple(all_in_names),
                out_names=tuple(out_names),
                lowering_input_output_aliases=(),
                sim_require_finite=True,
                sim_require_nnan=True,
                nc=nc,
            )
            return tuple(outs)

        devices = jax.devices()[:8]
        mesh = Mesh(np.asarray(devices), ("core",))
        self.mesh = mesh
        in_specs = (PartitionSpec("core"),) * (n_params + n_outs)
        out_specs = (PartitionSpec("core"),) * n_outs
        self.body = _body
        self.in_specs = in_specs
        self.out_specs = out_specs
        donate = tuple(range(n_params, n_params + n_outs))
        self.sharded = jax.jit(
            shard_map(
                _body,
                mesh=mesh,
                in_specs=in_specs,
                out_specs=out_specs,
                check_rep=False,
            ),
            donate_argnums=donate,
            keep_unused=True,
        )
        self.out_avals = out_avals

    def prepare(self, in_maps):
        """Concatenate per-core inputs along axis 0 (device-shardable)."""
        return [
            np.concatenate([in_maps[c][name] for c in range(8)], axis=0)
            for name in self.in_names
        ]

    def run(self, concat_in):
        zeros = [
            np.zeros((8 * z.shape[0], *z.shape[1:]), z.dtype) for z in self.zero_outs
        ]
        out_arrs = self.sharded(*concat_in, *zeros)
        res = []
        for c in range(8):
            res.append(
                {
                    name: np.asarray(out_arrs[i]).reshape(
                        8, *self.out_avals[i].shape
                    )[c]
                    for i, name in enumerate(self.out_names)
                }
            )
        return res


_RUNNER = None


def _get_runner():
    global _RUNNER
    if _RUNNER is None:
        _RUNNER = _Runner()
    return _RUNNER


def make_in_maps(inputs):
    f32 = lambda x: np.ascontiguousarray(np.asarray(x, dtype=np.float32))
    q, k, v = f32(inputs["q"]), f32(inputs["k"]), f32(inputs["v"])
    Wq, Wk, Wv, Wo = (f32(inputs[n]) for n in ("Wq", "Wk", "Wv", "Wo"))
    bq, bk, bv = (f32(inputs[n]) for n in ("bq", "bk", "bv"))
    in_maps = []
    for c in range(8):
        b, mh = c // 2, c % 2
        sl = slice(mh * M, (mh + 1) * M)
        in_maps.append(
            {
                "qb": q[b],
                "kb": np.ascontiguousarray(k[b, sl]),
                "vb": np.ascontiguousarray(v[b, sl]),
                "Wq": Wq, "Wk": Wk, "Wv": Wv, "Wo": Wo,
                "bq": bq, "bk": bk, "bv": bv,
            }
        )
    return in_maps


def assemble_out(results, bo):
    out = np.empty((B, N, HIDDEN), dtype=np.float32)
    for b in range(B):
        acc = results[2 * b]["outT"] + results[2 * b + 1]["outT"]
        out[b] = acc.T + bo[None, :]
    return out


def kernel(**inputs):
    runner = _get_runner()
    res = runner.run(runner.prepare(make_in_maps(inputs)))
    bo = np.asarray(inputs["bo"], dtype=np.float32)
    return assemble_out(res, bo)


# revision 22
# speedup vs baseline: 1.5239x; 1.1672x over previous
"""Trainium2 Bass kernel for nn_MultiHeadAttention_79706003079680.

Reference (fp32):
    qp = (q @ Wq + bq) * SCALE      # [B, N, PROJ]
    kp = k @ Wk + bk
    vp = v @ Wv + bv
    scores = einsum('bnd,bmd->bnm', qp, kp)
    attn = softmax(scores, axis=1)          # over the QUERY axis n
    x = einsum('bnm,bmd->bnd', attn, vp)
    out = x @ Wo + bo                       # [B, N, HIDDEN]

Sharding: 8 cores = 4 batches x 2 key-halves (m in [mh*1024, mh*1024+1024)).
Softmax over n couples all queries for a fixed key m, so each core keeps
all n=2048 queries and a slice of keys. Each core emits a partial
out^T [HIDDEN, N]; the host sums the two key-halves per batch, transposes,
and adds bo.

On-chip layout (per core, P=128 partitions):
  qT   [P, 4, 2048] f32    q^T (h on partitions), via PE transpose
  qp_T [32, P, 2048] fp16  staged in DRAM (d on partitions)
  kp_T [P, 32, 1024] fp16  resident (d on partitions)
  vp   [8, P, 4096] bf16   staged in DRAM (m on partitions)
  attn [P, 8, 2048] bf16   resident, normalized softmax weights

All big matmuls run at 1 PE cycle/row: float32r (fp32 truncated to FP22)
for the projections, fp16/bf16 for scores / x / out.
"""

import numpy as np

import concourse.bass as bass
import concourse.mybir as mybir
import concourse.tile as tile
from concourse.masks import make_identity

P = 128
HIDDEN = 512
NUM_HEADS = 8
PROJ = NUM_HEADS * HIDDEN          # 4096
B, N = 4, 2048
M = N // 2                         # keys per core = 1024
SCALE = (HIDDEN // NUM_HEADS) ** -0.5

HB = HIDDEN // P                   # 4 h-blocks of 128
DB = PROJ // P                     # 32 d-blocks of 128
NB = N // 512                      # 4 n-blocks of 512
MB = M // P                        # 8 m-blocks of 128

F32 = mybir.dt.float32
F32R = mybir.dt.float32r
F16 = mybir.dt.float16
BF16 = mybir.dt.bfloat16
AX = mybir.AxisListType.X
AF = mybir.ActivationFunctionType


MAX_WAITS = 1


def split_excess_waits(nc, max_waits=MAX_WAITS):
    """Move excess per-instruction sem waits onto same-engine NoOps.

    This walrus build rejects instructions carrying more than a couple of
    sync-wait commands ("Too many sync wait commands" in setupSyncWait).
    A NoOp placed immediately before the instruction on the same engine
    enforces the wait in program order with identical semantics.
    """
    n_extra = 0
    for f in nc.m.functions:
        for bb in f.blocks:
            insts = bb.instructions
            i = 0
            while i < len(insts):
                inst = insts[i]
                si = getattr(inst, "sync_info", None)
                if si is not None and si.on_wait and len(si.on_wait) > max_waits:
                    waits = list(si.on_wait)
                    si.on_wait = waits[: max_waits]
                    for w in waits[max_waits:]:
                        n_extra += 1
                        nop = mybir.InstNoOp(
                            name=f"I-wsplit{n_extra}",
                            ins=[],
                            outs=[],
                            engine=inst.engine,
                        )
                        nop.sync_info = mybir.SyncInfo(on_wait=[w], on_update=[])
                        try:
                            nc.register_instruction(nop)
                        except Exception:
                            pass
                        # insert immediately before inst (inst shifts right)
                        insts.insert(i, nop)
                        i += 1
                i += 1
    return n_extra


class PatchedTC(tile.TileContext):
    """TileContext that post-processes the module to satisfy this walrus
    build's per-instruction sync-wait limit."""

    def __exit__(self, exc_type, exc_val, exc_tb):
        ret = super().__exit__(exc_type, exc_val, exc_tb)
        if exc_type is None:
            split_excess_waits(self.nc)
        return ret


def r(ap):
    return ap.bitcast(F32R)


def _phase_a1(nc, tc, pst, psm, qb, Wq, Wo, qp_d, wo_d, bqs, ident):
    """q transpose + qp_T projection -> DRAM fp16; Wo cast -> DRAM fp16."""
    with (
        tc.tile_pool(name="wfull", bufs=1) as wfull,
        tc.tile_pool(name="actT", bufs=1) as actT,
        tc.tile_pool(name="ldp", bufs=3) as ldp,
        tc.tile_pool(name="stp", bufs=4) as stp,
    ):
        qT = actT.tile([P, HB, N], F32, tag="qT")
        wq_s = wfull.tile([P, HB, PROJ], F32, tag="w")
        wq_src = Wq.ap().rearrange("(hb p) d -> p hb d", p=P).bitcast(F32R)
        for nt in range(N // P):
            q_t = ldp.tile([P, HIDDEN], F32, tag="ld")
            nc.sync.dma_start(out=q_t, in_=qb[nt * P : (nt + 1) * P, :])
            for hb in range(HB):
                pt = pst.tile([P, P], F32, tag="tp")
                nc.tensor.transpose(pt, q_t[:, hb * P : (hb + 1) * P], ident)
                nc.vector.tensor_copy(qT[:, hb, nt * P : (nt + 1) * P].bitcast(F32R), pt.bitcast(F32R))
        # d-sliced weight loads: first quarter lands early so the first
        # projection matmuls start without waiting for the full 8MB
        DQ = PROJ // 4
        for dsl in range(4):
            nc.sync.dma_start(
                out=wq_s[:, :, dsl * DQ : (dsl + 1) * DQ].bitcast(F32R),
                in_=wq_src[:, :, dsl * DQ : (dsl + 1) * DQ],
            )
        for db in range(DB):
            for nb in range(NB):
                ps = psm.tile([P, 512], F32, tag="mm")
                for hb in range(HB):
                    nc.tensor.matmul(
                        ps,
                        r(wq_s[:, hb, db * P : (db + 1) * P]),
                        r(qT[:, hb, nb * 512 : (nb + 1) * 512]),
                        start=(hb == 0),
                        stop=(hb == HB - 1),
                    )
                st = stp.tile([P, 512], F16, tag="st")
                nc.scalar.activation(
                    st, ps, AF.Identity, bias=bqs[:, db : db + 1], scale=SCALE
                )
                nc.sync.dma_start(out=qp_d[db, :, nb * 512 : (nb + 1) * 512], in_=st)


def _phase_a23(nc, tc, pst, psm, kb, vb, Wk, Wv, bv, vp_d, kpT, bks, ident):
    """k/v transposes, kp_T projection -> SBUF fp16, vp -> DRAM bf16."""
    with (
        tc.tile_pool(name="wfull2", bufs=1) as wfull2,
        tc.tile_pool(name="actT2", bufs=1) as actT2,
        tc.tile_pool(name="ldp2", bufs=3) as ldp2,
        tc.tile_pool(name="stp2", bufs=4) as stp2,
        tc.tile_pool(name="brow", bufs=1) as brow,
    ):
        wk_s = wfull2.tile([P, HB, PROJ], F32, tag="w")
        wk_src = Wk.ap().rearrange("(hb p) d -> p hb d", p=P).bitcast(F32R)
        DQ = PROJ // 4
        for dsl in range(4):
            nc.sync.dma_start(
                out=wk_s[:, :, dsl * DQ : (dsl + 1) * DQ].bitcast(F32R),
                in_=wk_src[:, :, dsl * DQ : (dsl + 1) * DQ],
            )
        kT = actT2.tile([P, HB, M], F32, tag="aT")
        for mt in range(M // P):
            k_t = ldp2.tile([P, HIDDEN], F32, tag="ld")
            nc.sync.dma_start(out=k_t, in_=kb[mt * P : (mt + 1) * P, :])
            for hb in range(HB):
                pt = pst.tile([P, P], F32, tag="tp")
                nc.tensor.transpose(pt, k_t[:, hb * P : (hb + 1) * P], ident)
                nc.vector.tensor_copy(kT[:, hb, mt * P : (mt + 1) * P].bitcast(F32R), pt.bitcast(F32R))
        for db in range(DB):
            for m2 in range(M // 512):
                ps = psm.tile([P, 512], F32, tag="mm")
                for hb in range(HB):
                    nc.tensor.matmul(
                        ps,
                        r(wk_s[:, hb, db * P : (db + 1) * P]),
                        r(kT[:, hb, m2 * 512 : (m2 + 1) * 512]),
                        start=(hb == 0),
                        stop=(hb == HB - 1),
                    )
                nc.scalar.activation(
                    kpT[:, db, m2 * 512 : (m2 + 1) * 512],
                    ps,
                    AF.Identity,
                    bias=bks[:, db : db + 1],
                    scale=1.0,
                )

        bvrow = brow.tile([1, PROJ], F32)
        nc.sync.dma_start(out=bvrow.bitcast(F32R), in_=bv.ap().rearrange("(o a) -> o a", o=1).bitcast(F32R))
        ones_tmp = brow.tile([1, P], F32)
        nc.vector.memset(ones_tmp, 1.0)
        ones_row = brow.tile([1, P], F32)
        nc.vector.tensor_copy(ones_row.bitcast(F32R), ones_tmp.bitcast(F32R))
        vT = actT2.tile([P, HB, M], F32, tag="aT")
        for mt in range(M // P):
            v_t = ldp2.tile([P, HIDDEN], F32, tag="ld")
            nc.sync.dma_start(out=v_t, in_=vb[mt * P : (mt + 1) * P, :])
            for hb in range(HB):
                pt = pst.tile([P, P], F32, tag="tp")
                nc.tensor.transpose(pt, v_t[:, hb * P : (hb + 1) * P], ident)
                nc.vector.tensor_copy(vT[:, hb, mt * P : (mt + 1) * P].bitcast(F32R), pt.bitcast(F32R))
        wv_s = wfull2.tile([P, HB, PROJ], F32, tag="w")
        wv_src = Wv.ap().rearrange("(hb p) d -> p hb d", p=P).bitcast(F32R)
        for dsl in range(4):
            nc.sync.dma_start(
                out=wv_s[:, :, dsl * DQ : (dsl + 1) * DQ].bitcast(F32R),
                in_=wv_src[:, :, dsl * DQ : (dsl + 1) * DQ],
            )
        for mb in range(MB):
            for ds in range(PROJ // 512):
                ps = psm.tile([P, 512], F32, tag="mm")
                for hb in range(HB):
                    nc.tensor.matmul(
                        ps,
                        r(vT[:, hb, mb * P : (mb + 1) * P]),
                        r(wv_s[:, hb, ds * 512 : (ds + 1) * 512]),
                        start=(hb == 0),
                        stop=False,
                    )
                nc.tensor.matmul(
                    ps,
                    r(ones_row),
                    r(bvrow[:, ds * 512 : (ds + 1) * 512]),
                    start=False,
                    stop=True,
                )
                st = stp2.tile([P, 512], BF16, tag="st")
                nc.vector.tensor_copy(st, ps)
                nc.sync.dma_start(out=vp_d[mb, :, ds * 512 : (ds + 1) * 512], in_=st)


def _phase_b(nc, tc, psm, qp_d, kpT, attn, rsum, rinv, Wo, wo_d):
    """scores_T = kp_T^T(d) qp_T per m-block; softmax over free axis n.

    The Wo fp32->fp16 cast rides along here: phase B's DMA load is light
    (qp_s readback), so the 12MB of Wo traffic hides under the scores
    matmuls instead of competing with the startup q/Wq loads."""
    with (
        tc.tile_pool(name="qps", bufs=4) as qps,
        tc.tile_pool(name="scp", bufs=1) as scp,
        tc.tile_pool(name="nmx", bufs=4) as nmx,
        tc.tile_pool(name="wol", bufs=3) as wol,
        tc.tile_pool(name="wos", bufs=3) as wos,
    ):
        for db in range(DB):
            wo_t = wol.tile([P, HIDDEN], F32, tag="wl")
            nc.sync.dma_start(out=wo_t, in_=Wo[db * P : (db + 1) * P, :])
            wo_c = wos.tile([P, HIDDEN], F16, tag="ws")
            nc.vector.tensor_copy(wo_c, wo_t)
            nc.sync.dma_start(out=wo_d[db], in_=wo_c)
        scores = scp.tile([P, 4, N], F32)
        NC2 = N // 256
        for mh2 in range(2):
            for nc2 in range(NC2):
                qp_s = qps.tile([P, DB, 256], F16, tag="qp")
                nc.sync.dma_start(
                    out=qp_s,
                    in_=qp_d[:, :, nc2 * 256 : (nc2 + 1) * 256].rearrange(
                        "db p n -> p db n"
                    ),
                )
                for mbl in range(4):
                    mb = mh2 * 4 + mbl
                    ps = psm.tile([P, 512], F32, tag="mm")
                    for db in range(DB):
                        nc.tensor.matmul(
                            ps[:, 0:256],
                            kpT[:, db, mb * P : (mb + 1) * P],
                            qp_s[:, db, :],
                            start=(db == 0),
                            stop=(db == DB - 1),
                        )
                    nc.vector.tensor_copy(
                        scores[:, mbl, nc2 * 256 : (nc2 + 1) * 256], ps[:, 0:256]
                    )
            for mbl in range(4):
                mb = mh2 * 4 + mbl
                negmax = nmx.tile([P, 1], F32, tag="nm")
                nc.vector.reduce_max(negmax, scores[:, mbl, :], axis=AX, negate=True)
                nc.scalar.activation(
                    attn[:, mb, :],
                    scores[:, mbl, :],
                    AF.Exp,
                    bias=negmax,
                    scale=1.0,
                    accum_out=rsum[:, mb : mb + 1],
                )
                nc.vector.reciprocal(rinv[:, mb : mb + 1], rsum[:, mb : mb + 1])
                nc.vector.tensor_scalar_mul(
                    attn[:, mb, :], attn[:, mb, :], rinv[:, mb : mb + 1]
                )


def _phase_c(nc, tc, psm, vp_d, wo_d, attn, outT):
    """x_T = vp^T @ attn (per d-block), out_T = Wo^T @ x_T."""
    with (
        tc.tile_pool(name="vpc", bufs=1) as vpc,
        tc.tile_pool(name="woc", bufs=1) as woc,
        tc.tile_pool(name="xsp", bufs=2) as xsp,
        tc.tile_pool(name="osp", bufs=2) as osp,
    ):
        vp_c = vpc.tile([P, MB, PROJ], BF16)
        for mb in range(MB):
            nc.sync.dma_start(out=vp_c[:, mb, :], in_=vp_d[mb])
        wo16 = woc.tile([P, DB, HIDDEN], F16)
        nc.sync.dma_start(out=wo16, in_=wo_d.ap().rearrange("db p h -> p db h"))
        for nb in range(NB):
            x_s = xsp.tile([P, DB, 512], F16, tag="x")
            for db in range(DB):
                ps = psm.tile([P, 512], F32, tag="mm")
                for mch in range(MB):
                    nc.tensor.matmul(
                        ps,
                        vp_c[:, mch, db * P : (db + 1) * P],
                        attn[:, mch, nb * 512 : (nb + 1) * 512],
                        start=(mch == 0),
                        stop=(mch == MB - 1),
                    )
                nc.vector.tensor_copy(x_s[:, db, :], ps)
            for hb in range(HB):
                ps2 = psm.tile([P, 512], F32, tag="mm")
                for db in range(DB):
                    nc.tensor.matmul(
                        ps2,
                        wo16[:, db, hb * P : (hb + 1) * P],
                        x_s[:, db, :],
                        start=(db == 0),
                        stop=(db == DB - 1),
                    )
                ot = osp.tile([P, 512], F32, tag="ot")
                nc.vector.tensor_copy(ot, ps2)
                nc.sync.dma_start(
                    out=outT[hb * P : (hb + 1) * P, nb * 512 : (nb + 1) * 512],
                    in_=ot,
                )


def build_nc():
    nc = bass.Bass("TRN2", target_bir_lowering=False, debug=False, num_devices=8)

    qb = nc.dram_tensor("qb", [N, HIDDEN], F32, kind="ExternalInput")
    kb = nc.dram_tensor("kb", [M, HIDDEN], F32, kind="ExternalInput")
    vb = nc.dram_tensor("vb", [M, HIDDEN], F32, kind="ExternalInput")
    Wq = nc.dram_tensor("Wq", [HIDDEN, PROJ], F32, kind="ExternalInput")
    Wk = nc.dram_tensor("Wk", [HIDDEN, PROJ], F32, kind="ExternalInput")
    Wv = nc.dram_tensor("Wv", [HIDDEN, PROJ], F32, kind="ExternalInput")
    Wo = nc.dram_tensor("Wo", [PROJ, HIDDEN], F32, kind="ExternalInput")
    bq = nc.dram_tensor("bq", [PROJ], F32, kind="ExternalInput")
    bk = nc.dram_tensor("bk", [PROJ], F32, kind="ExternalInput")
    bv = nc.dram_tensor("bv", [PROJ], F32, kind="ExternalInput")
    outT = nc.dram_tensor("outT", [HIDDEN, N], F32, kind="ExternalOutput")

    qp_d = nc.dram_tensor("qp_d", [DB, P, N], F16, kind="Internal")
    vp_d = nc.dram_tensor("vp_d", [MB, P, PROJ], BF16, kind="Internal")
    wo_d = nc.dram_tensor("wo_d", [DB, P, HIDDEN], F16, kind="Internal")

    with PatchedTC(nc) as tc:
        with (
            tc.tile_pool(name="singles", bufs=1) as singles,
            tc.tile_pool(name="pst", bufs=4, space="PSUM") as pst,
            tc.tile_pool(name="psm", bufs=4, space="PSUM") as psm,
        ):
            ident = singles.tile([P, P], F32)
            make_identity(nc, ident)
            bqs = singles.tile([P, DB], F32)
            nc.sync.dma_start(out=bqs, in_=bq.ap().rearrange("(a b) -> b a", b=P))
            nc.scalar.mul(bqs, bqs, SCALE)
            bks = singles.tile([P, DB], F32)
            nc.sync.dma_start(out=bks, in_=bk.ap().rearrange("(a b) -> b a", b=P))
            rsum = singles.tile([P, MB], F32)
            rinv = singles.tile([P, MB], F32)

            _phase_a1(nc, tc, pst, psm, qb, Wq, Wo, qp_d, wo_d, bqs, ident)

            # attn outlives kpT: open its pool first, allocate lazily.
            with tc.tile_pool(name="attnp", bufs=1) as attnp:
                with tc.tile_pool(name="kpTp", bufs=1) as kpTp:
                    kpT = kpTp.tile([P, DB, M], F16)
                    _phase_a23(
                        nc, tc, pst, psm, kb, vb, Wk, Wv, bv, vp_d, kpT, bks, ident
                    )
                    attn = attnp.tile([P, MB, N], BF16)
                    _phase_b(nc, tc, psm, qp_d, kpT, attn, rsum, rinv, Wo, wo_d)
                _phase_c(nc, tc, psm, vp_d, wo_d, attn, outT)
    # A handful of waits are attached after the TileContext's own exit
    # processing; sweep again until the module is clean.
    while split_excess_waits(nc):
        pass
    return nc


class _Runner:
    """Compile the Bass program once; re-execute cheaply on later calls.

    Mirrors bass2jax.run_bass_via_pjrt's multi-core path, but keeps the
    jitted shard_map callable so repeated kernel() calls skip the
    multi-minute neuronxcc compile.
    """

    def __init__(self):
        import jax
        from jax.sharding import Mesh, PartitionSpec
        from jax.experimental.shard_map import shard_map
        from concourse import bass2jax
        import concourse.mybir as mb

        self.jax = jax
        nc = build_nc()
        self.nc = nc
        bass2jax.install_neuronx_cc_hook()

        in_names, out_names, out_avals, zero_outs = [], [], [], []
        partition_name = (
            nc.partition_id_tensor.name if nc.partition_id_tensor else None
        )
        for alloc in nc.m.functions[0].allocations:
            if not isinstance(alloc, mb.MemoryLocationSet):
                continue
            name = alloc.memorylocations[0].name
            if alloc.kind == "ExternalInput":
                if name != partition_name:
                    in_names.append(name)
            elif alloc.kind == "ExternalOutput":
                shape = tuple(alloc.tensor_shape)
                dtype = mb.dt.np(alloc.dtype)
                out_names.append(name)
                out_avals.append(jax.core.ShapedArray(shape, dtype))
                zero_outs.append(np.zeros(shape, dtype))
        n_params = len(in_names)
        n_outs = len(out_avals)
        all_in_names = list(in_names) + list(out_names)
        if partition_name is not None:
            all_in_names.append(partition_name)
        self.in_names = in_names
        self.out_names = out_names
        self.zero_outs = zero_outs

        def _body(*args):
            operands = list(args)
            if partition_name is not None:
                operands.append(bass2jax.partition_id_tensor())
            outs = bass2jax._bass_exec_p.bind(
                *operands,
                out_avals=tuple(out_avals),
                in_names=tuple(all_in_names),
                out_names=tuple(out_names),
                lowering_input_output_aliases=(),
                sim_require_finite=True,
                sim_require_nnan=True,
                nc=nc,
            )
            return tuple(outs)

        devices = jax.devices()[:8]
        mesh = Mesh(np.asarray(devices), ("core",))
        self.mesh = mesh
        in_specs = (PartitionSpec("core"),) * (n_params + n_outs)
        out_specs = (PartitionSpec("core"),) * n_outs
        self.body = _body
        self.in_specs = in_specs
        self.out_specs = out_specs
        donate = tuple(range(n_params, n_params + n_outs))
        self.sharded = jax.jit(
            shard_map(
                _body,
                mesh=mesh,
                in_specs=in_specs,
                out_specs=out_specs,
                check_rep=False,
            ),
            donate_argnums=donate,
            keep_unused=True,
        )
        self.out_avals = out_avals

    def prepare(self, in_maps):
        """Concatenate per-core inputs along axis 0 (device-shardable)."""
        return [
            np.concatenate([in_maps[c][name] for c in range(8)], axis=0)
            for name in self.in_names
        ]

    def run(self, concat_in):
        zeros = [
            np.zeros((8 * z.shape[0], *z.shape[1:]), z.dtype) for z in self.zero_outs
        ]
        out_arrs = self.sharded(*concat_in, *zeros)
        res = []
        for c in range(8):
            res.append(
                {
                    name: np.asarray(out_arrs[i]).reshape(
                        8, *self.out_avals[i].shape
                    )[c]
                    for i, name in enumerate(self.out_names)
                }
            )
        return res


_RUNNER = None


def _get_runner():
    global _RUNNER
    if _RUNNER is None:
        _RUNNER = _Runner()
    return _RUNNER


def make_in_maps(inputs):
    f32 = lambda x: np.ascontiguousarray(np.asarray(x, dtype=np.float32))
    q, k, v = f32(inputs["q"]), f32(inputs["k"]), f32(inputs["v"])
    Wq, Wk, Wv, Wo = (f32(inputs[n]) for n in ("Wq", "Wk", "Wv", "Wo"))
    bq, bk, bv = (f32(inputs[n]) for n in ("bq", "bk", "bv"))
    in_maps = []
    for c in range(8):
        b, mh = c // 2, c % 2
        sl = slice(mh * M, (mh + 1) * M)
        in_maps.append(
            {
                "qb": q[b],
                "kb": np.ascontiguousarray(k[b, sl]),
                "vb": np.ascontiguousarray(v[b, sl]),
                "Wq": Wq, "Wk": Wk, "Wv": Wv, "Wo": Wo,
                "bq": bq, "bk": bk, "bv": bv,
            }
        )
    return in_maps


def assemble_out(results, bo):
    out = np.empty((B, N, HIDDEN), dtype=np.float32)
    for b in range(B):
        acc = results[2 * b]["outT"] + results[2 * b + 1]["outT"]
        out[b] = acc.T + bo[None, :]
    return out


def kernel(**inputs):
    runner = _get_runner()
    res = runner.run(runner.prepare(make_in_maps(inputs)))
    bo = np.asarray(inputs["bo"], dtype=np.float32)
    return assemble_out(res, bo)


# revision 24
# speedup vs baseline: 1.6245x; 1.0661x over previous
"""Trainium2 Bass kernel for nn_MultiHeadAttention_79706003079680.

Reference (fp32):
    qp = (q @ Wq + bq) * SCALE      # [B, N, PROJ]
    kp = k @ Wk + bk
    vp = v @ Wv + bv
    scores = einsum('bnd,bmd->bnm', qp, kp)
    attn = softmax(scores, axis=1)          # over the QUERY axis n
    x = einsum('bnm,bmd->bnd', attn, vp)
    out = x @ Wo + bo                       # [B, N, HIDDEN]

Sharding: 8 cores = 4 batches x 2 key-halves (m in [mh*1024, mh*1024+1024)).
Softmax over n couples all queries for a fixed key m, so each core keeps
all n=2048 queries and a slice of keys. Each core emits a partial
out^T [HIDDEN, N]; the host sums the two key-halves per batch, transposes,
and adds bo.

On-chip layout (per core, P=128 partitions):
  qT   [P, 4, 2048] f32    q^T (h on partitions), via PE transpose
  qp_T [32, P, 2048] fp16  staged in DRAM (d on partitions)
  kp_T [P, 32, 1024] fp16  resident (d on partitions)
  vp   [8, P, 4096] bf16   staged in DRAM (m on partitions)
  attn [P, 8, 2048] bf16   resident, normalized softmax weights

All big matmuls run at 1 PE cycle/row: float32r (fp32 truncated to FP22)
for the projections, fp16/bf16 for scores / x / out.
"""

import numpy as np

import concourse.bass as bass
import concourse.mybir as mybir
import concourse.tile as tile
from concourse.masks import make_identity

P = 128
HIDDEN = 512
NUM_HEADS = 8
PROJ = NUM_HEADS * HIDDEN          # 4096
B, N = 4, 2048
M = N // 2                         # keys per core = 1024
SCALE = (HIDDEN // NUM_HEADS) ** -0.5

HB = HIDDEN // P                   # 4 h-blocks of 128
DB = PROJ // P                     # 32 d-blocks of 128
NB = N // 512                      # 4 n-blocks of 512
MB = M // P                        # 8 m-blocks of 128

F32 = mybir.dt.float32
F32R = mybir.dt.float32r
F16 = mybir.dt.float16
BF16 = mybir.dt.bfloat16
AX = mybir.AxisListType.X
AF = mybir.ActivationFunctionType


MAX_WAITS = 1


def split_excess_waits(nc, max_waits=MAX_WAITS):
    """Move excess per-instruction sem waits onto same-engine NoOps.

    This walrus build rejects instructions carrying more than a couple of
    sync-wait commands ("Too many sync wait commands" in setupSyncWait).
    A NoOp placed immediately before the instruction on the same engine
    enforces the wait in program order with identical semantics.
    """
    n_extra = 0
    for f in nc.m.functions:
        for bb in f.blocks:
            insts = bb.instructions
            i = 0
            while i < len(insts):
                inst = insts[i]
                si = getattr(inst, "sync_info", None)
                if si is not None and si.on_wait and len(si.on_wait) > max_waits:
                    waits = list(si.on_wait)
                    si.on_wait = waits[: max_waits]
                    for w in waits[max_waits:]:
                        n_extra += 1
                        nop = mybir.InstNoOp(
                            name=f"I-wsplit{n_extra}",
                            ins=[],
                            outs=[],
                            engine=inst.engine,
                        )
                        nop.sync_info = mybir.SyncInfo(on_wait=[w], on_update=[])
                        try:
                            nc.register_instruction(nop)
                        except Exception:
                            pass
                        # insert immediately before inst (inst shifts right)
                        insts.insert(i, nop)
                        i += 1
                i += 1
    return n_extra


class PatchedTC(tile.TileContext):
    """TileContext that post-processes the module to satisfy this walrus
    build's per-instruction sync-wait limit."""

    def __exit__(self, exc_type, exc_val, exc_tb):
        ret = super().__exit__(exc_type, exc_val, exc_tb)
        if exc_type is None:
            split_excess_waits(self.nc)
        return ret


def r(ap):
    return ap.bitcast(F32R)


def _phase_a1(nc, tc, pst, psm, qb, Wq, Wo, qp_d, wo_d, bqs, ident):
    """q transpose + qp_T projection -> DRAM fp16; Wo cast -> DRAM fp16."""
    with (
        tc.tile_pool(name="wfull", bufs=1) as wfull,
        tc.tile_pool(name="actT", bufs=1) as actT,
        tc.tile_pool(name="ldp", bufs=3) as ldp,
        tc.tile_pool(name="stp", bufs=4) as stp,
    ):
        qT = actT.tile([P, HB, N], F32, tag="qT")
        wq_s = wfull.tile([P, HB, PROJ], F32, tag="w")
        wq_src = Wq.ap().rearrange("(hb p) d -> p hb d", p=P).bitcast(F32R)
        for nt in range(N // P):
            q_t = ldp.tile([P, HIDDEN], F32, tag="ld")
            nc.sync.dma_start(out=q_t, in_=qb[nt * P : (nt + 1) * P, :])
            for hb in range(HB):
                pt = pst.tile([P, P], F32, tag="tp")
                nc.tensor.transpose(pt, q_t[:, hb * P : (hb + 1) * P], ident)
                nc.vector.tensor_copy(qT[:, hb, nt * P : (nt + 1) * P].bitcast(F32R), pt.bitcast(F32R))
        # d-sliced weight loads: first quarter lands early so the first
        # projection matmuls start without waiting for the full 8MB
        DQ = PROJ // 4
        for dsl in range(4):
            nc.sync.dma_start(
                out=wq_s[:, :, dsl * DQ : (dsl + 1) * DQ].bitcast(F32R),
                in_=wq_src[:, :, dsl * DQ : (dsl + 1) * DQ],
            )
        for db in range(DB):
            for nb in range(NB):
                ps = psm.tile([P, 512], F32, tag="mm")
                for hb in range(HB):
                    nc.tensor.matmul(
                        ps,
                        r(wq_s[:, hb, db * P : (db + 1) * P]),
                        r(qT[:, hb, nb * 512 : (nb + 1) * 512]),
                        start=(hb == 0),
                        stop=(hb == HB - 1),
                    )
                st = stp.tile([P, 512], F16, tag="st")
                nc.scalar.activation(
                    st, ps, AF.Identity, bias=bqs[:, db : db + 1], scale=SCALE
                )
                nc.sync.dma_start(out=qp_d[db, :, nb * 512 : (nb + 1) * 512], in_=st)


def _phase_a23(nc, tc, pst, psm, kb, vb, Wk, Wv, bv, vp_d, kpT, bks, ident):
    """k/v transposes, kp_T projection -> SBUF fp16, vp -> DRAM bf16."""
    with (
        tc.tile_pool(name="wfull2", bufs=1) as wfull2,
        tc.tile_pool(name="actT2", bufs=1) as actT2,
        tc.tile_pool(name="ldp2", bufs=3) as ldp2,
        tc.tile_pool(name="stp2", bufs=4) as stp2,
        tc.tile_pool(name="brow", bufs=1) as brow,
    ):
        wk_s = wfull2.tile([P, HB, PROJ], F32, tag="w")
        wk_src = Wk.ap().rearrange("(hb p) d -> p hb d", p=P).bitcast(F32R)
        DQ = PROJ // 4
        for dsl in range(4):
            nc.sync.dma_start(
                out=wk_s[:, :, dsl * DQ : (dsl + 1) * DQ].bitcast(F32R),
                in_=wk_src[:, :, dsl * DQ : (dsl + 1) * DQ],
            )
        kT = actT2.tile([P, HB, M], F32, tag="aT")
        for mt in range(M // P):
            k_t = ldp2.tile([P, HIDDEN], F32, tag="ld")
            nc.sync.dma_start(out=k_t, in_=kb[mt * P : (mt + 1) * P, :])
            for hb in range(HB):
                pt = pst.tile([P, P], F32, tag="tp")
                nc.tensor.transpose(pt, k_t[:, hb * P : (hb + 1) * P], ident)
                nc.vector.tensor_copy(kT[:, hb, mt * P : (mt + 1) * P].bitcast(F32R), pt.bitcast(F32R))
        for db in range(DB):
            for m2 in range(M // 512):
                ps = psm.tile([P, 512], F32, tag="mm")
                for hb in range(HB):
                    nc.tensor.matmul(
                        ps,
                        r(wk_s[:, hb, db * P : (db + 1) * P]),
                        r(kT[:, hb, m2 * 512 : (m2 + 1) * 512]),
                        start=(hb == 0),
                        stop=(hb == HB - 1),
                    )
                nc.scalar.activation(
                    kpT[:, db, m2 * 512 : (m2 + 1) * 512],
                    ps,
                    AF.Identity,
                    bias=bks[:, db : db + 1],
                    scale=1.0,
                )

        bvrow = brow.tile([1, PROJ], F32)
        nc.sync.dma_start(out=bvrow.bitcast(F32R), in_=bv.ap().rearrange("(o a) -> o a", o=1).bitcast(F32R))
        ones_tmp = brow.tile([1, P], F32)
        nc.vector.memset(ones_tmp, 1.0)
        ones_row = brow.tile([1, P], F32)
        nc.vector.tensor_copy(ones_row.bitcast(F32R), ones_tmp.bitcast(F32R))
        vT = actT2.tile([P, HB, M], F32, tag="aT")
        for mt in range(M // P):
            v_t = ldp2.tile([P, HIDDEN], F32, tag="ld")
            nc.sync.dma_start(out=v_t, in_=vb[mt * P : (mt + 1) * P, :])
            for hb in range(HB):
                pt = pst.tile([P, P], F32, tag="tp")
                nc.tensor.transpose(pt, v_t[:, hb * P : (hb + 1) * P], ident)
                nc.vector.tensor_copy(vT[:, hb, mt * P : (mt + 1) * P].bitcast(F32R), pt.bitcast(F32R))
        wv_s = wfull2.tile([P, HB, PROJ], F32, tag="w")
        wv_src = Wv.ap().rearrange("(hb p) d -> p hb d", p=P).bitcast(F32R)
        for dsl in range(4):
            nc.sync.dma_start(
                out=wv_s[:, :, dsl * DQ : (dsl + 1) * DQ].bitcast(F32R),
                in_=wv_src[:, :, dsl * DQ : (dsl + 1) * DQ],
            )
        for mb in range(MB):
            for ds in range(PROJ // 512):
                ps = psm.tile([P, 512], F32, tag="mm")
                for hb in range(HB):
                    nc.tensor.matmul(
                        ps,
                        r(vT[:, hb, mb * P : (mb + 1) * P]),
                        r(wv_s[:, hb, ds * 512 : (ds + 1) * 512]),
                        start=(hb == 0),
                        stop=False,
                    )
                nc.tensor.matmul(
                    ps,
                    r(ones_row),
                    r(bvrow[:, ds * 512 : (ds + 1) * 512]),
                    start=False,
                    stop=True,
                )
                st = stp2.tile([P, 512], BF16, tag="st")
                nc.vector.tensor_copy(st, ps)
                nc.sync.dma_start(out=vp_d[mb, :, ds * 512 : (ds + 1) * 512], in_=st)


def _phase_b(nc, tc, psm, qp_d, kpT, attn, rsum, rinv, Wo, wo_d):
    """scores_T = kp_T^T(d) qp_T per m-block; softmax over free axis n.

    The Wo fp32->fp16 cast rides along here: phase B's DMA load is light
    (qp_s readback), so the 12MB of Wo traffic hides under the scores
    matmuls instead of competing with the startup q/Wq loads."""
    with (
        tc.tile_pool(name="qps", bufs=4) as qps,
        tc.tile_pool(name="scp", bufs=1) as scp,
        tc.tile_pool(name="nmx", bufs=4) as nmx,
        tc.tile_pool(name="wol", bufs=3) as wol,
        tc.tile_pool(name="wos", bufs=3) as wos,
    ):
        for db in range(DB):
            wo_t = wol.tile([P, HIDDEN], F32, tag="wl")
            nc.sync.dma_start(out=wo_t, in_=Wo[db * P : (db + 1) * P, :])
            wo_c = wos.tile([P, HIDDEN], F16, tag="ws")
            nc.vector.tensor_copy(wo_c, wo_t)
            nc.sync.dma_start(out=wo_d[db], in_=wo_c)
        scores = scp.tile([P, 4, N], F32)
        NC2 = N // 256
        for mh2 in range(2):
            for nc2 in range(NC2):
                qp_s = qps.tile([P, DB, 256], F16, tag="qp")
                nc.sync.dma_start(
                    out=qp_s,
                    in_=qp_d[:, :, nc2 * 256 : (nc2 + 1) * 256].rearrange(
                        "db p n -> p db n"
                    ),
                )
                for mbl in range(4):
                    mb = mh2 * 4 + mbl
                    ps = psm.tile([P, 512], F32, tag="mm")
                    for db in range(DB):
                        nc.tensor.matmul(
                            ps[:, 0:256],
                            kpT[:, db, mb * P : (mb + 1) * P],
                            qp_s[:, db, :],
                            start=(db == 0),
                            stop=(db == DB - 1),
                        )
                    nc.vector.tensor_copy(
                        scores[:, mbl, nc2 * 256 : (nc2 + 1) * 256], ps[:, 0:256]
                    )
            for mbl in range(4):
                mb = mh2 * 4 + mbl
                negmax = nmx.tile([P, 1], F32, tag="nm")
                nc.vector.reduce_max(negmax, scores[:, mbl, :], axis=AX, negate=True)
                nc.scalar.activation(
                    attn[:, mb, :],
                    scores[:, mbl, :],
                    AF.Exp,
                    bias=negmax,
                    scale=1.0,
                    accum_out=rsum[:, mb : mb + 1],
                )
                nc.vector.reciprocal(rinv[:, mb : mb + 1], rsum[:, mb : mb + 1])
                nc.vector.tensor_scalar_mul(
                    attn[:, mb, :], attn[:, mb, :], rinv[:, mb : mb + 1]
                )


def _phase_c(nc, tc, psm, vp_d, wo_d, attn, outT):
    """x_T = vp^T @ attn (per d-block), out_T = Wo^T @ x_T."""
    with (
        tc.tile_pool(name="vpc", bufs=1) as vpc,
        tc.tile_pool(name="woc", bufs=1) as woc,
        tc.tile_pool(name="xsp", bufs=2) as xsp,
        tc.tile_pool(name="osp", bufs=2) as osp,
    ):
        vp_c = vpc.tile([P, MB, PROJ], BF16)
        for mb in range(MB):
            nc.sync.dma_start(out=vp_c[:, mb, :], in_=vp_d[mb])
        wo16 = woc.tile([P, DB, HIDDEN], F16)
        nc.sync.dma_start(out=wo16, in_=wo_d.ap().rearrange("db p h -> p db h"))
        for nb in range(NB):
            x_s = xsp.tile([P, DB, 512], F16, tag="x")
            for db in range(DB):
                ps = psm.tile([P, 512], F32, tag="mm")
                for mch in range(MB):
                    nc.tensor.matmul(
                        ps,
                        vp_c[:, mch, db * P : (db + 1) * P],
                        attn[:, mch, nb * 512 : (nb + 1) * 512],
                        start=(mch == 0),
                        stop=(mch == MB - 1),
                    )
                nc.vector.tensor_copy(x_s[:, db, :], ps)
            for hb in range(HB):
                ps2 = psm.tile([P, 512], F32, tag="mm")
                for db in range(DB):
                    nc.tensor.matmul(
                        ps2,
                        wo16[:, db, hb * P : (hb + 1) * P],
                        x_s[:, db, :],
                        start=(db == 0),
                        stop=(db == DB - 1),
                    )
                ot = osp.tile([P, 512], F32, tag="ot")
                nc.vector.tensor_copy(ot, ps2)
                nc.sync.dma_start(
                    out=outT[hb * P : (hb + 1) * P, nb * 512 : (nb + 1) * 512],
                    in_=ot,
                )


def build_nc():
    nc = bass.Bass("TRN2", target_bir_lowering=False, debug=False, num_devices=8)

    qb = nc.dram_tensor("qb", [N, HIDDEN], F32, kind="ExternalInput")
    kb = nc.dram_tensor("kb", [M, HIDDEN], F32, kind="ExternalInput")
    vb = nc.dram_tensor("vb", [M, HIDDEN], F32, kind="ExternalInput")
    Wq = nc.dram_tensor("Wq", [HIDDEN, PROJ], F32, kind="ExternalInput")
    Wk = nc.dram_tensor("Wk", [HIDDEN, PROJ], F32, kind="ExternalInput")
    Wv = nc.dram_tensor("Wv", [HIDDEN, PROJ], F32, kind="ExternalInput")
    Wo = nc.dram_tensor("Wo", [PROJ, HIDDEN], F32, kind="ExternalInput")
    bq = nc.dram_tensor("bq", [PROJ], F32, kind="ExternalInput")
    bk = nc.dram_tensor("bk", [PROJ], F32, kind="ExternalInput")
    bv = nc.dram_tensor("bv", [PROJ], F32, kind="ExternalInput")
    outT = nc.dram_tensor("outT", [HIDDEN, N], F32, kind="ExternalOutput")

    qp_d = nc.dram_tensor("qp_d", [DB, P, N], F16, kind="Internal")
    vp_d = nc.dram_tensor("vp_d", [MB, P, PROJ], BF16, kind="Internal")
    wo_d = nc.dram_tensor("wo_d", [DB, P, HIDDEN], F16, kind="Internal")

    with PatchedTC(nc) as tc:
        with (
            tc.tile_pool(name="singles", bufs=1) as singles,
            tc.tile_pool(name="pst", bufs=4, space="PSUM") as pst,
            tc.tile_pool(name="psm", bufs=4, space="PSUM") as psm,
        ):
            ident = singles.tile([P, P], F32)
            make_identity(nc, ident)
            # biases need (p, db) layout with d inner on partitions; a direct
            # strided DMA would be 4096 4-byte descriptors, so load the
            # contiguous [DB, P] view and PE-transpose it instead.
            bq_raw = singles.tile([DB, P], F32)
            nc.sync.dma_start(out=bq_raw, in_=bq.ap().rearrange("(a b) -> a b", b=P))
            bqs = singles.tile([P, DB], F32)
            ptb = pst.tile([P, DB], F32, tag="tp")
            nc.tensor.transpose(ptb, bq_raw, ident[:DB, :DB])
            nc.scalar.activation(bqs, ptb, AF.Identity, scale=SCALE)
            bk_raw = singles.tile([DB, P], F32)
            nc.sync.dma_start(out=bk_raw, in_=bk.ap().rearrange("(a b) -> a b", b=P))
            bks = singles.tile([P, DB], F32)
            ptb2 = pst.tile([P, DB], F32, tag="tp")
            nc.tensor.transpose(ptb2, bk_raw, ident[:DB, :DB])
            nc.vector.tensor_copy(bks, ptb2)
            rsum = singles.tile([P, MB], F32)
            rinv = singles.tile([P, MB], F32)

            _phase_a1(nc, tc, pst, psm, qb, Wq, Wo, qp_d, wo_d, bqs, ident)

            # attn outlives kpT: open its pool first, allocate lazily.
            with tc.tile_pool(name="attnp", bufs=1) as attnp:
                with tc.tile_pool(name="kpTp", bufs=1) as kpTp:
                    kpT = kpTp.tile([P, DB, M], F16)
                    _phase_a23(
                        nc, tc, pst, psm, kb, vb, Wk, Wv, bv, vp_d, kpT, bks, ident
                    )
                    attn = attnp.tile([P, MB, N], BF16)
                    _phase_b(nc, tc, psm, qp_d, kpT, attn, rsum, rinv, Wo, wo_d)
                _phase_c(nc, tc, psm, vp_d, wo_d, attn, outT)
    # A handful of waits are attached after the TileContext's own exit
    # processing; sweep again until the module is clean.
    while split_excess_waits(nc):
        pass
    return nc


class _Runner:
    """Compile the Bass program once; re-execute cheaply on later calls.

    Mirrors bass2jax.run_bass_via_pjrt's multi-core path, but keeps the
    jitted shard_map callable so repeated kernel() calls skip the
    multi-minute neuronxcc compile.
    """

    def __init__(self):
        import jax
        from jax.sharding import Mesh, PartitionSpec
        from jax.experimental.shard_map import shard_map
        from concourse import bass2jax
        import concourse.mybir as mb

        self.jax = jax
        nc = build_nc()
        self.nc = nc
        bass2jax.install_neuronx_cc_hook()

        in_names, out_names, out_avals, zero_outs = [], [], [], []
        partition_name = (
            nc.partition_id_tensor.name if nc.partition_id_tensor else None
        )
        for alloc in nc.m.functions[0].allocations:
            if not isinstance(alloc, mb.MemoryLocationSet):
                continue
            name = alloc.memorylocations[0].name
            if alloc.kind == "ExternalInput":
                if name != partition_name:
                    in_names.append(name)
            elif alloc.kind == "ExternalOutput":
                shape = tuple(alloc.tensor_shape)
                dtype = mb.dt.np(alloc.dtype)
                out_names.append(name)
                out_avals.append(jax.core.ShapedArray(shape, dtype))
                zero_outs.append(np.zeros(shape, dtype))
        n_params = len(in_names)
        n_outs = len(out_avals)
        all_in_names = list(in_names) + list(out_names)
        if partition_name is not None:
            all_in_names.append(partition_name)
        self.in_names = in_names
        self.out_names = out_names
        self.zero_outs = zero_outs

        def _body(*args):
            operands = list(args)
            if partition_name is not None:
                operands.append(bass2jax.partition_id_tensor())
            outs = bass2jax._bass_exec_p.bind(
                *operands,
                out_avals=tuple(out_avals),
                in_names=tuple(all_in_names),
                out_names=tuple(out_names),
                lowering_input_output_aliases=(),
                sim_require_finite=True,
                sim_require_nnan=True,
                nc=nc,
            )
            return tuple(outs)

        devices = jax.devices()[:8]
        mesh = Mesh(np.asarray(devices), ("core",))
        self.mesh = mesh
        in_specs = (PartitionSpec("core"),) * (n_params + n_outs)
        out_specs = (PartitionSpec("core"),) * n_outs
        self.body = _body
        self.in_specs = in_specs
        self.out_specs = out_specs
        donate = tuple(range(n_params, n_params + n_outs))
        self.sharded = jax.jit(
            shard_map(
                _body,
                mesh=mesh,
                in_specs=in_specs,
                out_specs=out_specs,
                check_rep=False,
            ),
            donate_argnums=donate,
            keep_unused=True,
        )
        self.out_avals = out_avals

    def prepare(self, in_maps):
        """Concatenate per-core inputs along axis 0 (device-shardable)."""
        return [
            np.concatenate([in_maps[c][name] for c in range(8)], axis=0)
            for name in self.in_names
        ]

    def run(self, concat_in):
        zeros = [
            np.zeros((8 * z.shape[0], *z.shape[1:]), z.dtype) for z in self.zero_outs
        ]
        out_arrs = self.sharded(*concat_in, *zeros)
        res = []
        for c in range(8):
            res.append(
                {
                    name: np.asarray(out_arrs[i]).reshape(
                        8, *self.out_avals[i].shape
                    )[c]
                    for i, name in enumerate(self.out_names)
                }
            )
        return res


_RUNNER = None


def _get_runner():
    global _RUNNER
    if _RUNNER is None:
        _RUNNER = _Runner()
    return _RUNNER


def make_in_maps(inputs):
    f32 = lambda x: np.ascontiguousarray(np.asarray(x, dtype=np.float32))
    q, k, v = f32(inputs["q"]), f32(inputs["k"]), f32(inputs["v"])
    Wq, Wk, Wv, Wo = (f32(inputs[n]) for n in ("Wq", "Wk", "Wv", "Wo"))
    bq, bk, bv = (f32(inputs[n]) for n in ("bq", "bk", "bv"))
    in_maps = []
    for c in range(8):
        b, mh = c // 2, c % 2
        sl = slice(mh * M, (mh + 1) * M)
        in_maps.append(
            {
                "qb": q[b],
                "kb": np.ascontiguousarray(k[b, sl]),
                "vb": np.ascontiguousarray(v[b, sl]),
                "Wq": Wq, "Wk": Wk, "Wv": Wv, "Wo": Wo,
                "bq": bq, "bk": bk, "bv": bv,
            }
        )
    return in_maps


def assemble_out(results, bo):
    out = np.empty((B, N, HIDDEN), dtype=np.float32)
    for b in range(B):
        acc = results[2 * b]["outT"] + results[2 * b + 1]["outT"]
        out[b] = acc.T + bo[None, :]
    return out


def kernel(**inputs):
    runner = _get_runner()
    res = runner.run(runner.prepare(make_in_maps(inputs)))
    bo = np.asarray(inputs["bo"], dtype=np.float32)
    return assemble_out(res, bo)
